# revision 23
# baseline (speedup 1.0000x reference)
"""Causal dense attention (key=value) on 8 TRN2 NeuronCores.

Reference semantics (B=4, T=2048, D=1024, fp32):
    scores  = Q @ V^T                      [B, T, T]
    scores -= 1e9 * (~tril)                causal mask
    W       = softmax(scores, axis=-1)
    out     = W @ V                        [B, T, D]

Sharding: 2 cores per batch. Each batch's 16 causal q-tiles (128 rows
each, kv extent 128*(t+1)) are split odd/even so both cores get the
same padded kv-extent schedule EXT = [256, 512, ..., 2048] (ascending),
making the Bass program identical across all 8 cores (pure SPMD).
Padding columns are killed by the additive causal mask.

Host stages per core: Q^T (d-major), V^T (d-major), V (natural, bf16),
additive masks for the last 256 columns of each slot. The device kernel
runs flash-style softmax per 128-row q-tile with the full score strip
held in PSUM windows.
"""

import numpy as np

import concourse.bass as bass
import concourse.mybir as mybir
from concourse import bacc, tile
from concourse.bass_utils import run_bass_kernel_spmd
from concourse.masks import make_identity

import ml_dtypes

B, T, D = 4, 2048, 1024
NCORES = 8
NSLOT = 8
EXT = [256 * (j + 1) for j in range(NSLOT)]  # kv extent per slot
# processing order: small slot first (fast start), smallest slot last (short
# pipeline tail); big slots in the middle keep the PE pipeline saturated
ORDER = [1, 2, 3, 4, 5, 6, 7, 0]
NEG_INF = 1e9

F32 = mybir.dt.float32
F32R = mybir.dt.float32r
BF16 = mybir.dt.bfloat16

# matmul dtypes (tuning knobs)
MM1_DT = F32R   # scores matmul
MM2_BF16 = True  # weights/value matmul in bf16


def _round_f32r(x):
    """Round fp32 to 11 mantissa bits (RNE) — matches walrus fp32_to_fp32r."""
    u = np.asarray(x, dtype=np.float32).view(np.uint32)
    u = u + 0x7FF + ((u >> 12) & 1)
    u &= np.uint32(0xFFFFF000)
    return u.view(np.float32)


def _tiles_for_core(c):
    """q-tile index (within the batch) for each slot, for core c."""
    if c < 4:
        return [2 * j + 1 for j in range(NSLOT)]  # extents exactly EXT
    return [2 * j for j in range(NSLOT)]  # extents EXT - 128 (padded)


def _build_program():
    nc = bacc.Bacc("TRN2", target_bir_lowering=False)

    qt_d = nc.dram_tensor("qt", [D, NSLOT * 128], MM1_DT, kind="ExternalInput")
    vt_d = nc.dram_tensor("vt", [D, T], MM1_DT, kind="ExternalInput")
    v_d = nc.dram_tensor(
        "v", [T, D], BF16 if MM2_BF16 else F32, kind="ExternalInput"
    )
    mask_d = nc.dram_tensor("mask", [NSLOT, 128, 256], BF16, kind="ExternalInput")
    o_d = nc.dram_tensor("o", [NSLOT * 128, D], F32, kind="ExternalOutput")

    v_dt = BF16 if MM2_BF16 else F32

    with tile.TileContext(nc) as tc:
        with (
            tc.tile_pool(name="const", bufs=1) as constp,
            tc.tile_pool(name="qt", bufs=1) as qtp,
            tc.tile_pool(name="vt", bufs=1) as vtp,
            tc.tile_pool(name="vn", bufs=1) as vnp,
            tc.tile_pool(name="w", bufs=2) as wp,
            tc.tile_pool(name="wt", bufs=4) as wtp,
            tc.tile_pool(name="osb", bufs=2) as op,
            tc.tile_pool(name="stats", bufs=24) as statp,
            tc.tile_pool(name="ps_s", bufs=2, space="PSUM") as ps_s,
            tc.tile_pool(name="ps_t", bufs=2, space="PSUM") as ps_t,
            tc.tile_pool(name="ps_o", bufs=1, space="PSUM") as ps_o,
        ):
            ident = constp.tile([128, 128], BF16, tag="ident")
            make_identity(nc, ident[:])

            # ACT exp-table warm-up: load exp_and_others during initial DMAs
            warm = statp.tile([128, 1], F32, tag="warm")
            nc.gpsimd.memset(warm[:], 0.0)
            nc.scalar.activation(warm[:], warm[:], mybir.ActivationFunctionType.Exp)

            # Resident inputs, DMA'd in slot-consumption order (per ORDER):
            # each slot first needs its Q^T quarter, V^T chunks and V rows up
            # to its extent; the masks ride along after the first slot's data.
            masks = constp.tile([128, NSLOT * 256], BF16, tag="masks")
            qts = {}   # (d8, j) -> [128, 128] AP
            vts = {}   # (d8, kc) -> [128, 512]
            vns = {}   # kb -> [128, D]
            masks_emitted = False
            for oi, j in enumerate(ORDER):
                q4 = j // 4
                if (0, j) not in qts:
                    for d8 in range(8):
                        t_ = qtp.tile([128, 512], MM1_DT, tag=f"qt{d8}_{q4}")
                        nc.sync.dma_start(
                            t_[:],
                            qt_d[
                                d8 * 128 : (d8 + 1) * 128,
                                q4 * 512 : (q4 + 1) * 512,
                            ],
                        )
                        for jj in range(4 * q4, 4 * q4 + 4):
                            qts[(d8, jj)] = t_[
                                :, (jj % 4) * 128 : (jj % 4 + 1) * 128
                            ]
                for kc in range((EXT[j] + 511) // 512):
                    if (0, kc) in vts:
                        continue
                    for d8 in range(8):
                        t_ = vtp.tile([128, 512], MM1_DT, tag=f"vt{d8}_{kc}")
                        nc.sync.dma_start(
                            t_[:],
                            vt_d[
                                d8 * 128 : (d8 + 1) * 128,
                                kc * 512 : (kc + 1) * 512,
                            ],
                        )
                        vts[(d8, kc)] = t_
                for kb in range(EXT[j] // 128):
                    if kb in vns:
                        continue
                    t_ = vnp.tile([128, D], v_dt, tag=f"vn{kb}")
                    nc.sync.dma_start(t_[:], v_d[kb * 128 : (kb + 1) * 128, :])
                    vns[kb] = t_
                if not masks_emitted:
                    for jj in range(NSLOT):
                        nc.sync.dma_start(
                            masks[:, jj * 256 : (jj + 1) * 256], mask_d[jj, :, :]
                        )
                    masks_emitted = True

            w_dt = BF16 if MM2_BF16 else F32

            def emit_front(j):
                """mm1 + softmax stats + exp for slot j. Returns (j, w_sb, rinv)."""
                E = EXT[j]
                # kilo-windows of up to 1024 columns (each a [128,1024] PSUM
                # tile = 2 banks; matmuls still issue per 512-column bank)
                kws = [(c0, min(1024, E - c0)) for c0 in range(0, E, 1024)]

                s_tiles = []
                for c0, kwd in kws:
                    s_ = ps_s.tile([128, 1024], F32, tag="sw")
                    for h0 in range(0, kwd, 512):
                        hw = min(512, kwd - h0)
                        a0 = c0 + h0  # absolute column
                        for d8 in range(8):
                            nc.tensor.matmul(
                                s_[:, h0 : h0 + hw],
                                qts[(d8, j)],
                                vts[(d8, a0 // 512)][
                                    :, a0 % 512 : a0 % 512 + hw
                                ],
                                start=(d8 == 0),
                                stop=(d8 == 7),
                            )
                    s_tiles.append(s_)

                # causal mask on the last 256 columns
                lc0, lwd = kws[-1]
                moff = lwd - 256
                nc.vector.tensor_add(
                    s_tiles[-1][:, moff : moff + 256],
                    s_tiles[-1][:, moff : moff + 256],
                    masks[:, j * 256 : (j + 1) * 256],
                )

                # negated row max over the strip
                nmax = None
                for ki, (c0, kwd) in enumerate(kws):
                    nm = statp.tile([128, 1], F32, tag="nm")
                    nc.vector.reduce_max(
                        nm[:], s_tiles[ki][:, :kwd], axis=mybir.AxisListType.X,
                        negate=True,
                    )
                    if nmax is None:
                        nmax = nm
                    else:
                        nm2 = statp.tile([128, 1], F32, tag="nmc")
                        nc.vector.tensor_tensor(
                            nm2[:], nmax[:], nm[:], op=mybir.AluOpType.min
                        )
                        nmax = nm2

                # exp (fused bias & row-sum) -> W in SBUF
                w_sb = wp.tile([128, E], w_dt, tag="w")
                rsum = None
                for ki, (c0, kwd) in enumerate(kws):
                    rs = statp.tile([128, 1], F32, tag="rs")
                    nc.scalar.activation(
                        w_sb[:, c0 : c0 + kwd],
                        s_tiles[ki][:, :kwd],
                        mybir.ActivationFunctionType.Exp,
                        bias=nmax[:],
                        accum_out=rs[:],
                    )
                    if rsum is None:
                        rsum = rs
                    else:
                        rs2 = statp.tile([128, 1], F32, tag="rsc")
                        nc.vector.tensor_add(rs2[:], rsum[:], rs[:])
                        rsum = rs2
                rinv = statp.tile([128, 1], F32, tag="rinv")
                nc.vector.reciprocal(rinv[:], rsum[:])
                return (j, w_sb, rinv)

            def emit_back(state):
                """transpose W, mm2, normalize, store for a finished slot."""
                j, w_sb, rinv = state
                E = EXT[j]
                nblk = E // 128
                wt_tiles = []
                for g0 in range(0, nblk, 4):
                    gn = min(4, nblk - g0)
                    t_ps = ps_t.tile([128, 512], w_dt, tag="tp")
                    for bi in range(gn):
                        blk = g0 + bi
                        nc.tensor.transpose(
                            t_ps[:, bi * 128 : (bi + 1) * 128],
                            w_sb[:, blk * 128 : (blk + 1) * 128],
                            ident[:],
                        )
                    wt_sb = wtp.tile([128, 512], w_dt, tag="wt")
                    nc.any.tensor_copy(wt_sb[:, : gn * 128], t_ps[:, : gn * 128])
                    wt_tiles.append(wt_sb)

                # output matmul: O[q, d] = W^T.T @ V
                o_ps = ps_o.tile([128, D], F32, tag="op")
                for blk in range(nblk):
                    wt_ap = wt_tiles[blk // 4][
                        :, (blk % 4) * 128 : (blk % 4 + 1) * 128
                    ]
                    v_ap = vns[blk]
                    if not MM2_BF16:
                        wt_ap = wt_ap.bitcast(F32R)
                    for dd in range(0, D, 512):
                        va = v_ap[:, dd : dd + 512]
                        if not MM2_BF16:
                            va = va.bitcast(F32R)
                        nc.tensor.matmul(
                            o_ps[:, dd : dd + 512],
                            wt_ap,
                            va,
                            start=(blk == 0),
                            stop=(blk == nblk - 1),
                        )

                o_sb = op.tile([128, D], F32, tag="o")
                nc.scalar.activation(
                    o_sb[:],
                    o_ps[:],
                    mybir.ActivationFunctionType.Copy,
                    scale=rinv[:],
                )
                nc.sync.dma_start(o_d[j * 128 : (j + 1) * 128, :], o_sb[:])

            # software pipeline: slot j's scores/softmax overlap the previous
            # slot's transpose+mm2 on the PE
            pending = None
            for j in ORDER:
                st = emit_front(j)
                if pending is not None:
                    emit_back(pending)
                pending = st
            emit_back(pending)

    nc.finalize()
    return nc


_NC_CACHE = None


def _get_program():
    global _NC_CACHE
    if _NC_CACHE is None:
        _NC_CACHE = _build_program()
    return _NC_CACHE


def stage_inputs(query, value):
    """Build the 8 per-core input maps from the full query/value arrays."""
    query = np.asarray(query, dtype=np.float32)
    value = np.asarray(value, dtype=np.float32)

    in_maps = []
    for c in range(NCORES):
        b = c % 4
        tiles = _tiles_for_core(c)

        q_rows = np.concatenate(
            [query[b, t * 128 : (t + 1) * 128, :] for t in tiles], axis=0
        )  # [1024, D]
        qt = np.ascontiguousarray(q_rows.T)  # [D, 1024]
        vt = np.ascontiguousarray(value[b].T)  # [D, T]
        if MM1_DT == F32R:
            qt = _round_f32r(qt)
            vt = _round_f32r(vt)
        vn = value[b]
        if MM2_BF16:
            vn = vn.astype(ml_dtypes.bfloat16)
        vn = np.ascontiguousarray(vn)

        mask = np.zeros((NSLOT, 128, 256), dtype=np.float32)
        for j in range(NSLOT):
            t = tiles[j]
            rows = t * 128 + np.arange(128)[:, None]  # global q row
            cols = EXT[j] - 256 + np.arange(256)[None, :]  # global kv col
            mask[j][cols > rows] = -NEG_INF
        mask = mask.astype(ml_dtypes.bfloat16)

        in_maps.append({"qt": qt, "vt": vt, "v": vn, "mask": mask})
    return in_maps


def kernel(query, value):
    nc = _get_program()
    in_maps = stage_inputs(query, value)
    res = run_bass_kernel_spmd(nc, in_maps, core_ids=list(range(NCORES)))

    out = np.empty((B, T, D), dtype=np.float32)
    for c in range(NCORES):
        o = res.results[c]["o"]  # [1024, D]
        b = c % 4
        for j, t in enumerate(_tiles_for_core(c)):
            out[b, t * 128 : (t + 1) * 128, :] = o[j * 128 : (j + 1) * 128, :]
    return out


# revision 27
# speedup vs baseline: 10.5535x; 10.5535x over previous
"""Causal dense attention (key=value) on 8 TRN2 NeuronCores.

Reference semantics (B=4, T=2048, D=1024, fp32):
    scores  = Q @ V^T                      [B, T, T]
    scores -= 1e9 * (~tril)                causal mask
    W       = softmax(scores, axis=-1)
    out     = W @ V                        [B, T, D]

Sharding: 2 cores per batch. Each batch's 16 causal q-tiles (128 rows
each, kv extent 128*(t+1)) are split odd/even so both cores get the
same padded kv-extent schedule EXT = [256, 512, ..., 2048] (ascending),
making the Bass program identical across all 8 cores (pure SPMD).
Padding columns are killed by the additive causal mask.

Host stages per core: Q^T (d-major), V^T (d-major), V (natural, bf16),
additive masks for the last 256 columns of each slot. The device kernel
runs flash-style softmax per 128-row q-tile with the full score strip
held in PSUM windows.
"""

import numpy as np

import concourse.bass as bass
import concourse.mybir as mybir
from concourse import bacc, tile
from concourse.bass_utils import run_bass_kernel_spmd
from concourse.masks import make_identity

import ml_dtypes

B, T, D = 4, 2048, 1024
NCORES = 8
NSLOT = 8
EXT = [256 * (j + 1) for j in range(NSLOT)]  # kv extent per slot
# processing order: small slot first (fast start), smallest slot last (short
# pipeline tail); big slots in the middle keep the PE pipeline saturated
ORDER = [1, 2, 3, 4, 5, 6, 7, 0]
NEG_INF = 1e9

F32 = mybir.dt.float32
F32R = mybir.dt.float32r
BF16 = mybir.dt.bfloat16

# matmul dtypes (tuning knobs)
MM1_DT = F32R   # scores matmul
MM2_BF16 = True  # weights/value matmul in bf16
REPEAT = 1      # >1: repeat the whole pipeline in-program (bench only)


def _round_f32r(x):
    """Round fp32 to 11 mantissa bits (RNE) — matches walrus fp32_to_fp32r."""
    u = np.asarray(x, dtype=np.float32).view(np.uint32)
    u = u + 0x7FF + ((u >> 12) & 1)
    u &= np.uint32(0xFFFFF000)
    return u.view(np.float32)


def _tiles_for_core(c):
    """q-tile index (within the batch) for each slot, for core c."""
    if c < 4:
        return [2 * j + 1 for j in range(NSLOT)]  # extents exactly EXT
    return [2 * j for j in range(NSLOT)]  # extents EXT - 128 (padded)


def _build_program():
    nc = bacc.Bacc("TRN2", target_bir_lowering=False)

    qt_d = nc.dram_tensor("qt", [D, NSLOT * 128], MM1_DT, kind="ExternalInput")
    vt_d = nc.dram_tensor("vt", [D, T], MM1_DT, kind="ExternalInput")
    v_d = nc.dram_tensor(
        "v", [T, D], BF16 if MM2_BF16 else F32, kind="ExternalInput"
    )
    mask_d = nc.dram_tensor("mask", [NSLOT, 128, 256], BF16, kind="ExternalInput")
    o_d = nc.dram_tensor("o", [NSLOT * 128, D], F32, kind="ExternalOutput")

    v_dt = BF16 if MM2_BF16 else F32

    with tile.TileContext(nc) as tc:
        with (
            tc.tile_pool(name="const", bufs=1) as constp,
            tc.tile_pool(name="qt", bufs=1) as qtp,
            tc.tile_pool(name="vt", bufs=1) as vtp,
            tc.tile_pool(name="vn", bufs=1) as vnp,
            tc.tile_pool(name="w", bufs=2) as wp,
            tc.tile_pool(name="wt", bufs=4) as wtp,
            tc.tile_pool(name="osb", bufs=2) as op,
            tc.tile_pool(name="stats", bufs=24) as statp,
            tc.tile_pool(name="ps_s", bufs=2, space="PSUM") as ps_s,
            tc.tile_pool(name="ps_t", bufs=2, space="PSUM") as ps_t,
            tc.tile_pool(name="ps_o", bufs=1, space="PSUM") as ps_o,
        ):
            ident = constp.tile([128, 128], BF16, tag="ident")
            make_identity(nc, ident[:])

            # ACT exp-table warm-up: load exp_and_others during initial DMAs
            warm = statp.tile([128, 1], F32, tag="warm")
            nc.gpsimd.memset(warm[:], 0.0)
            nc.scalar.activation(warm[:], warm[:], mybir.ActivationFunctionType.Exp)

            masks = constp.tile([128, NSLOT * 256], BF16, tag="masks")
            w_dt = BF16 if MM2_BF16 else F32

            # Resident inputs, DMA'd in slot-consumption order (per ORDER):
            # each slot first needs its Q^T quarter, V^T chunks and V rows up
            # to its extent; the masks ride along after the first slot's data.
            qts = {}   # (d8, j) -> [128, 128] AP
            vts = {}   # (d8, kc) -> [128, 512]
            vns = {}   # kb -> [128, D]

            def emit_dma_waves(j, first_rep):
                q2 = j // 2
                if (0, j) not in qts:
                    for d8 in range(8):
                        t_ = qtp.tile([128, 256], MM1_DT, tag=f"qt{d8}_{q2}")
                        nc.sync.dma_start(
                            t_[:],
                            qt_d[
                                d8 * 128 : (d8 + 1) * 128,
                                q2 * 256 : (q2 + 1) * 256,
                            ],
                        )
                        for jj in range(2 * q2, 2 * q2 + 2):
                            qts[(d8, jj)] = t_[
                                :, (jj % 2) * 128 : (jj % 2 + 1) * 128
                            ]
                for kc in range((EXT[j] + 511) // 512):
                    if (0, kc) in vts:
                        continue
                    for d8 in range(8):
                        t_ = vtp.tile([128, 512], MM1_DT, tag=f"vt{d8}_{kc}")
                        nc.sync.dma_start(
                            t_[:],
                            vt_d[
                                d8 * 128 : (d8 + 1) * 128,
                                kc * 512 : (kc + 1) * 512,
                            ],
                        )
                        vts[(d8, kc)] = t_
                for kb in range(EXT[j] // 128):
                    if kb in vns:
                        continue
                    t_ = vnp.tile([128, D], v_dt, tag=f"vn{kb}")
                    nc.sync.dma_start(t_[:], v_d[kb * 128 : (kb + 1) * 128, :])
                    vns[kb] = t_
                if first_rep and j == ORDER[0]:
                    for jj in range(NSLOT):
                        nc.sync.dma_start(
                            masks[:, jj * 256 : (jj + 1) * 256], mask_d[jj, :, :]
                        )

            def emit_front(j):
                """mm1 + softmax stats + exp for slot j. Returns (j, w_sb, rinv)."""
                E = EXT[j]
                # kilo-windows of up to 1024 columns (each a [128,1024] PSUM
                # tile = 2 banks; matmuls still issue per 512-column bank)
                kws = [(c0, min(1024, E - c0)) for c0 in range(0, E, 1024)]

                s_tiles = []
                for c0, kwd in kws:
                    s_ = ps_s.tile([128, 1024], F32, tag="sw")
                    for h0 in range(0, kwd, 512):
                        hw = min(512, kwd - h0)
                        a0 = c0 + h0  # absolute column
                        for d8 in range(8):
                            nc.tensor.matmul(
                                s_[:, h0 : h0 + hw],
                                qts[(d8, j)],
                                vts[(d8, a0 // 512)][
                                    :, a0 % 512 : a0 % 512 + hw
                                ],
                                start=(d8 == 0),
                                stop=(d8 == 7),
                            )
                    s_tiles.append(s_)

                # causal mask on the last 256 columns
                lc0, lwd = kws[-1]
                moff = lwd - 256
                nc.vector.tensor_add(
                    s_tiles[-1][:, moff : moff + 256],
                    s_tiles[-1][:, moff : moff + 256],
                    masks[:, j * 256 : (j + 1) * 256],
                )

                # negated row max over the strip
                nmax = None
                for ki, (c0, kwd) in enumerate(kws):
                    nm = statp.tile([128, 1], F32, tag="nm")
                    nc.vector.reduce_max(
                        nm[:], s_tiles[ki][:, :kwd], axis=mybir.AxisListType.X,
                        negate=True,
                    )
                    if nmax is None:
                        nmax = nm
                    else:
                        nm2 = statp.tile([128, 1], F32, tag="nmc")
                        nc.vector.tensor_tensor(
                            nm2[:], nmax[:], nm[:], op=mybir.AluOpType.min
                        )
                        nmax = nm2

                # exp (fused bias & row-sum) -> W in SBUF
                w_sb = wp.tile([128, E], w_dt, tag="w")
                rsum = None
                for ki, (c0, kwd) in enumerate(kws):
                    rs = statp.tile([128, 1], F32, tag="rs")
                    nc.scalar.activation(
                        w_sb[:, c0 : c0 + kwd],
                        s_tiles[ki][:, :kwd],
                        mybir.ActivationFunctionType.Exp,
                        bias=nmax[:],
                        accum_out=rs[:],
                    )
                    if rsum is None:
                        rsum = rs
                    else:
                        rs2 = statp.tile([128, 1], F32, tag="rsc")
                        nc.vector.tensor_add(rs2[:], rsum[:], rs[:])
                        rsum = rs2
                rinv = statp.tile([128, 1], F32, tag="rinv")
                nc.vector.reciprocal(rinv[:], rsum[:])
                return (j, w_sb, rinv)

            def emit_back(state):
                """transpose W, mm2, normalize, store for a finished slot."""
                j, w_sb, rinv = state
                E = EXT[j]
                nblk = E // 128
                wt_tiles = []
                for g0 in range(0, nblk, 4):
                    gn = min(4, nblk - g0)
                    t_ps = ps_t.tile([128, 512], w_dt, tag="tp")
                    for bi in range(gn):
                        blk = g0 + bi
                        nc.tensor.transpose(
                            t_ps[:, bi * 128 : (bi + 1) * 128],
                            w_sb[:, blk * 128 : (blk + 1) * 128],
                            ident[:],
                        )
                    wt_sb = wtp.tile([128, 512], w_dt, tag="wt")
                    nc.any.tensor_copy(wt_sb[:, : gn * 128], t_ps[:, : gn * 128])
                    wt_tiles.append(wt_sb)

                # output matmul: O[q, d] = W^T.T @ V
                o_ps = ps_o.tile([128, D], F32, tag="op")
                for blk in range(nblk):
                    wt_ap = wt_tiles[blk // 4][
                        :, (blk % 4) * 128 : (blk % 4 + 1) * 128
                    ]
                    v_ap = vns[blk]
                    if not MM2_BF16:
                        wt_ap = wt_ap.bitcast(F32R)
                    for dd in range(0, D, 512):
                        va = v_ap[:, dd : dd + 512]
                        if not MM2_BF16:
                            va = va.bitcast(F32R)
                        nc.tensor.matmul(
                            o_ps[:, dd : dd + 512],
                            wt_ap,
                            va,
                            start=(blk == 0),
                            stop=(blk == nblk - 1),
                        )

                o_sb = op.tile([128, D], F32, tag="o")
                nc.scalar.activation(
                    o_sb[:],
                    o_ps[:],
                    mybir.ActivationFunctionType.Copy,
                    scale=rinv[:],
                )
                nc.sync.dma_start(o_d[j * 128 : (j + 1) * 128, :], o_sb[:])

            # software pipeline: slot j's scores/softmax overlap the previous
            # slot's transpose+mm2 on the PE
            pending = None
            for rep in range(REPEAT):
                if rep > 0:
                    qts.clear()
                    vts.clear()
                    vns.clear()
                for j in ORDER:
                    emit_dma_waves(j, rep == 0)
                    st = emit_front(j)
                    if pending is not None:
                        emit_back(pending)
                    pending = st
            emit_back(pending)

    nc.finalize()
    return nc


_NC_CACHE = None


def _get_program():
    global _NC_CACHE
    if _NC_CACHE is None:
        _NC_CACHE = _build_program()
    return _NC_CACHE


def stage_inputs(query, value):
    """Build the 8 per-core input maps from the full query/value arrays."""
    query = np.asarray(query, dtype=np.float32)
    value = np.asarray(value, dtype=np.float32)

    in_maps = []
    for c in range(NCORES):
        b = c % 4
        tiles = _tiles_for_core(c)

        q_rows = np.concatenate(
            [query[b, t * 128 : (t + 1) * 128, :] for t in tiles], axis=0
        )  # [1024, D]
        qt = np.ascontiguousarray(q_rows.T)  # [D, 1024]
        vt = np.ascontiguousarray(value[b].T)  # [D, T]
        if MM1_DT == F32R:
            qt = _round_f32r(qt)
            vt = _round_f32r(vt)
        vn = value[b]
        if MM2_BF16:
            vn = vn.astype(ml_dtypes.bfloat16)
        vn = np.ascontiguousarray(vn)

        mask = np.zeros((NSLOT, 128, 256), dtype=np.float32)
        for j in range(NSLOT):
            t = tiles[j]
            rows = t * 128 + np.arange(128)[:, None]  # global q row
            cols = EXT[j] - 256 + np.arange(256)[None, :]  # global kv col
            mask[j][cols > rows] = -NEG_INF
        mask = mask.astype(ml_dtypes.bfloat16)

        in_maps.append({"qt": qt, "vt": vt, "v": vn, "mask": mask})
    return in_maps


def kernel(query, value):
    nc = _get_program()
    in_maps = stage_inputs(query, value)
    res = run_bass_kernel_spmd(nc, in_maps, core_ids=list(range(NCORES)))

    out = np.empty((B, T, D), dtype=np.float32)
    for c in range(NCORES):
        o = res.results[c]["o"]  # [1024, D]
        b = c % 4
        for j, t in enumerate(_tiles_for_core(c)):
            out[b, t * 128 : (t + 1) * 128, :] = o[j * 128 : (j + 1) * 128, :]
    return out


# revision 51
# speedup vs baseline: 107.1479x; 10.1529x over previous
"""Causal dense attention (key=value) on 8 TRN2 NeuronCores.

Reference semantics (B=4, T=2048, D=1024, fp32):
    scores  = Q @ V^T                      [B, T, T]
    scores -= 1e9 * (~tril)                causal mask
    W       = softmax(scores, axis=-1)
    out     = W @ V                        [B, T, D]

Sharding: 2 cores per batch. Each batch's 16 causal q-tiles (128 rows
each, kv extent 128*(t+1)) are split odd/even so both cores get the
same padded kv-extent schedule EXT = [256, 512, ..., 2048] (ascending),
making the Bass program identical across all 8 cores (pure SPMD).
Padding columns are killed by the additive causal mask.

Host stages per core: Q^T (d-major) and V^T (d-major) pre-rounded to
f32r (full-rate TensorE), V (natural, bf16), and additive causal masks
for the last 256 columns of each slot.

Device pipeline per slot (software-pipelined, lag 2):
  mm1  S = Q^T.T @ V^T into PSUM kilo-windows (f32r), with the causal
       mask folded into the accumulation group as an identity-weighted
       matmul (S += I.T @ mask);
  stats row-max (DVE reduce, negated) -> exp with fused bias and
       accumulated row-sum (ScalarE) -> W (bf16) in SBUF;
  mm2  PE-transpose of W blocks, W^T.T @ V (bf16) into PSUM, then a
       ScalarE copy fused with the 1/rowsum scale, and DMA out.
Input DMAs are coalesced (one strided descriptor-set per wave) and
ordered by slot consumption so compute starts ~2 MiB in.
"""

import numpy as np

import concourse.bass as bass
import concourse.mybir as mybir
from concourse import bacc, tile
from concourse.bass_utils import run_bass_kernel_spmd
from concourse.masks import make_identity

import ml_dtypes

B, T, D = 4, 2048, 1024
NCORES = 8
NSLOT = 8
EXT = [256 * (j + 1) for j in range(NSLOT)]  # kv extent per slot
# processing order: smallest slot first (fast start), small slot last (short
# pipeline tail); big slots in the middle keep the PE pipeline saturated
ORDER = [1, 2, 3, 4, 5, 6, 7, 0]
NEG_INF = 1e9

F32 = mybir.dt.float32
F32R = mybir.dt.float32r
BF16 = mybir.dt.bfloat16

# matmul dtypes (tuning knobs)
MM1_DT = F32R   # scores matmul
MM2_BF16 = True  # weights/value matmul in bf16
REPEAT = 1      # >1: repeat the whole pipeline in-program (bench only)


def _round_f32r(x):
    """Round fp32 to 11 mantissa bits (RNE) — matches walrus fp32_to_fp32r."""
    u = np.asarray(x, dtype=np.float32).view(np.uint32)
    u = u + 0x7FF + ((u >> 12) & 1)
    u &= np.uint32(0xFFFFF000)
    return u.view(np.float32)


def _tiles_for_core(c):
    """q-tile index (within the batch) for each slot, for core c."""
    if c < 4:
        return [2 * j + 1 for j in range(NSLOT)]  # extents exactly EXT
    return [2 * j for j in range(NSLOT)]  # extents EXT - 128 (padded)


def _build_program():
    nc = bacc.Bacc("TRN2", target_bir_lowering=False)

    qt_d = nc.dram_tensor("qt", [D, NSLOT * 128], MM1_DT, kind="ExternalInput")
    vt_d = nc.dram_tensor("vt", [D, T], MM1_DT, kind="ExternalInput")
    v_d = nc.dram_tensor(
        "v", [T, D], BF16 if MM2_BF16 else F32, kind="ExternalInput"
    )
    mask_d = nc.dram_tensor("mask", [NSLOT, 128, 256], MM1_DT, kind="ExternalInput")
    identr_d = nc.dram_tensor("identr", [128, 128], MM1_DT, kind="ExternalInput")
    o_d = nc.dram_tensor("o", [NSLOT * 128, D], F32, kind="ExternalOutput")

    v_dt = BF16 if MM2_BF16 else F32

    with tile.TileContext(nc) as tc:
        with (
            tc.tile_pool(name="const", bufs=1) as constp,
            tc.tile_pool(name="qt", bufs=1) as qtp,
            tc.tile_pool(name="vt", bufs=1) as vtp,
            tc.tile_pool(name="vn", bufs=1) as vnp,
            tc.tile_pool(name="w", bufs=3) as wp,
            tc.tile_pool(name="wt", bufs=8) as wtp,
            tc.tile_pool(name="osb", bufs=2) as op,
            tc.tile_pool(name="stats", bufs=24) as statp,
            tc.tile_pool(name="ps_s", bufs=2, space="PSUM") as ps_s,
            tc.tile_pool(name="ps_t", bufs=2, space="PSUM") as ps_t,
            tc.tile_pool(name="ps_o", bufs=1, space="PSUM") as ps_o,
        ):
            ident = constp.tile([128, 128], BF16, tag="ident")
            make_identity(nc, ident[:])
            ident_r = constp.tile([128, 128], MM1_DT, tag="identr")
            nc.sync.dma_start(ident_r[:], identr_d[:])

            # ACT exp-table warm-up: load exp_and_others during initial DMAs
            warm = statp.tile([128, 1], F32, tag="warm")
            nc.gpsimd.memset(warm[:], 0.0)
            nc.scalar.activation(warm[:], warm[:], mybir.ActivationFunctionType.Exp)

            masks = constp.tile([128, NSLOT * 256], MM1_DT, tag="masks")
            w_dt = BF16 if MM2_BF16 else F32

            # Resident inputs, DMA'd in slot-consumption order (per ORDER):
            # each slot first needs its Q^T quarter, V^T chunks and V rows up
            # to its extent; the masks ride along after the first slot's data.
            qts = {}   # (d8, j) -> [128, 128] AP
            vts = {}   # (d8, kc) -> [128, 512]
            vns = {}   # kb -> [128, D]

            def emit_dma_waves(j, first_rep):
                # one coalesced DMA per wave: SBUF [128, 8, w] <- DRAM
                # [(8*128), w] with d8 stacked along the free dim
                if (0, j) not in qts:
                    # early waves small (fast pipeline start), later 512-wide
                    if j < 2:
                        c0, cw = 0, 256
                    elif j < 4:
                        c0, cw = 256, 256
                    else:
                        c0, cw = 512, 512
                    t_ = qtp.tile([128, 8, cw], MM1_DT, tag=f"qtw{c0}")
                    nc.sync.dma_start(
                        t_[:], qt_d[:, c0 : c0 + cw].rearrange("(a p) q -> p a q", p=128)
                    )
                    for d8 in range(8):
                        for jj in range(c0 // 128, (c0 + cw) // 128):
                            qts[(d8, jj)] = t_[
                                :, d8,
                                (jj - c0 // 128) * 128 : (jj - c0 // 128 + 1) * 128,
                            ]
                for kc in range((EXT[j] + 511) // 512):
                    if (0, kc) in vts:
                        continue
                    t_ = vtp.tile([128, 8, 512], MM1_DT, tag=f"vtw{kc}")
                    if kc == 0:
                        # split the first chunk so the opening slot's
                        # matmuls start after ~1MiB instead of 2MiB
                        for hh in (0, 256):
                            nc.sync.dma_start(
                                t_[:, :, hh : hh + 256],
                                vt_d[:, hh : hh + 256].rearrange(
                                    "(a p) k -> p a k", p=128
                                ),
                            )
                    else:
                        nc.sync.dma_start(
                            t_[:],
                            vt_d[:, kc * 512 : (kc + 1) * 512].rearrange(
                                "(a p) k -> p a k", p=128
                            ),
                        )
                    for d8 in range(8):
                        vts[(d8, kc)] = t_[:, d8, :]
                if first_rep and j == ORDER[0]:
                    nc.sync.dma_start(
                        masks[:].rearrange("p (j c) -> p j c", j=NSLOT),
                        mask_d.rearrange("j p c -> p j c"),
                    )

            def emit_front(j):
                """mm1 + softmax stats + exp for slot j. Returns (j, w_sb, rinv)."""
                E = EXT[j]
                # kilo-windows of up to 1024 columns (each a [128,1024] PSUM
                # tile = 2 banks; matmuls still issue per 512-column bank)
                kws = [(c0, min(1024, E - c0)) for c0 in range(0, E, 1024)]

                s_tiles = []
                for c0, kwd in kws:
                    s_ = ps_s.tile([128, 1024], F32, tag="sw")
                    for h0 in range(0, kwd, 512):
                        hw = min(512, kwd - h0)
                        a0 = c0 + h0  # absolute column
                        # the additive causal mask (last 256 columns of the
                        # slot) rides the accumulation group as an extra
                        # identity-weighted matmul: S[q,c] += I[k,q]*mask[k,c]
                        last_half = c0 + h0 + hw == E
                        for d8 in range(8):
                            nc.tensor.matmul(
                                s_[:, h0 : h0 + hw],
                                qts[(d8, j)],
                                vts[(d8, a0 // 512)][
                                    :, a0 % 512 : a0 % 512 + hw
                                ],
                                start=(d8 == 0),
                                stop=(d8 == 7 and not last_half),
                            )
                        if last_half:
                            nc.tensor.matmul(
                                s_[:, h0 + hw - 256 : h0 + hw],
                                ident_r[:],
                                masks[:, j * 256 : (j + 1) * 256],
                                start=False,
                                stop=True,
                            )
                    s_tiles.append(s_)

                # negated row max over the strip
                nmax = None
                for ki, (c0, kwd) in enumerate(kws):
                    nm = statp.tile([128, 1], F32, tag="nm")
                    nc.vector.reduce_max(
                        nm[:], s_tiles[ki][:, :kwd], axis=mybir.AxisListType.X,
                        negate=True,
                    )
                    if nmax is None:
                        nmax = nm
                    else:
                        nm2 = statp.tile([128, 1], F32, tag="nmc")
                        nc.vector.tensor_tensor(
                            nm2[:], nmax[:], nm[:], op=mybir.AluOpType.min
                        )
                        nmax = nm2

                # exp (fused bias & row-sum) -> W in SBUF
                w_sb = wp.tile([128, E], w_dt, tag="w")
                rsum = None
                for ki, (c0, kwd) in enumerate(kws):
                    rs = statp.tile([128, 1], F32, tag="rs")
                    nc.scalar.activation(
                        w_sb[:, c0 : c0 + kwd],
                        s_tiles[ki][:, :kwd],
                        mybir.ActivationFunctionType.Exp,
                        bias=nmax[:],
                        accum_out=rs[:],
                    )
                    if rsum is None:
                        rsum = rs
                    else:
                        rs2 = statp.tile([128, 1], F32, tag="rsc")
                        nc.vector.tensor_add(rs2[:], rsum[:], rs[:])
                        rsum = rs2
                rinv = statp.tile([128, 1], F32, tag="rinv")
                nc.vector.reciprocal(rinv[:], rsum[:])
                return (j, w_sb, rinv)

            def emit_vn_waves(j):
                for q_ in range((EXT[j] // 128 + 3) // 4):
                    if q_ * 4 in vns:
                        continue
                    t_ = vnp.tile([128, 4, D], v_dt, tag=f"vnw{q_}")
                    nc.sync.dma_start(
                        t_[:],
                        v_d[q_ * 512 : (q_ + 1) * 512, :].rearrange(
                            "(a p) d -> p a d", p=128
                        ),
                    )
                    for kb in range(q_ * 4, q_ * 4 + 4):
                        vns[kb] = t_[:, kb % 4, :]

            def emit_back(state):
                """transpose W, mm2, normalize, store for a finished slot."""
                j, w_sb, rinv = state
                E = EXT[j]
                nblk = E // 128
                wt_tiles = []
                for g0 in range(0, nblk, 4):
                    gn = min(4, nblk - g0)
                    t_ps = ps_t.tile([128, 512], w_dt, tag="tp")
                    for bi in range(gn):
                        blk = g0 + bi
                        nc.tensor.transpose(
                            t_ps[:, bi * 128 : (bi + 1) * 128],
                            w_sb[:, blk * 128 : (blk + 1) * 128],
                            ident[:],
                        )
                    wt_sb = wtp.tile([128, 512], w_dt, tag="wt")
                    nc.vector.tensor_copy(wt_sb[:, : gn * 128], t_ps[:, : gn * 128])
                    wt_tiles.append(wt_sb)

                # output matmul: O[q, d] = W^T.T @ V
                o_ps = ps_o.tile([128, D], F32, tag="op")
                for blk in range(nblk):
                    wt_ap = wt_tiles[blk // 4][
                        :, (blk % 4) * 128 : (blk % 4 + 1) * 128
                    ]
                    v_ap = vns[blk]
                    if not MM2_BF16:
                        wt_ap = wt_ap.bitcast(F32R)
                    for dd in range(0, D, 512):
                        va = v_ap[:, dd : dd + 512]
                        if not MM2_BF16:
                            va = va.bitcast(F32R)
                        nc.tensor.matmul(
                            o_ps[:, dd : dd + 512],
                            wt_ap,
                            va,
                            start=(blk == 0),
                            stop=(blk == nblk - 1),
                        )

                o_sb = op.tile([128, D], F32, tag="o")
                nc.scalar.activation(
                    o_sb[:],
                    o_ps[:],
                    mybir.ActivationFunctionType.Copy,
                    scale=rinv[:],
                )
                nc.sync.dma_start(o_d[j * 128 : (j + 1) * 128, :], o_sb[:])

            # software pipeline (lag 2): slot j's scores/softmax overlap the
            # two previous slots' transpose+mm2 work queued on the PE
            pending = []
            for rep in range(REPEAT):
                if rep > 0:
                    # drain the pipeline before re-loading inputs (bench path)
                    for st in pending:
                        emit_back(st)
                    pending = []
                    qts.clear()
                    vts.clear()
                    vns.clear()
                for j in ORDER:
                    emit_dma_waves(j, rep == 0)
                    pending.append(emit_front(j))
                    emit_vn_waves(j)
                    if len(pending) > 2:
                        emit_back(pending.pop(0))
            for st in pending:
                emit_back(st)

    nc.finalize()
    return nc


_NC_CACHE = None


def _get_program():
    global _NC_CACHE
    if _NC_CACHE is None:
        _NC_CACHE = _build_program()
    return _NC_CACHE


def stage_inputs(query, value):
    """Build the 8 per-core input maps from the full query/value arrays."""
    query = np.asarray(query, dtype=np.float32)
    value = np.asarray(value, dtype=np.float32)

    in_maps = []
    for c in range(NCORES):
        b = c % 4
        tiles = _tiles_for_core(c)

        q_rows = np.concatenate(
            [query[b, t * 128 : (t + 1) * 128, :] for t in tiles], axis=0
        )  # [1024, D]
        qt = np.ascontiguousarray(q_rows.T)  # [D, 1024]
        vt = np.ascontiguousarray(value[b].T)  # [D, T]
        if MM1_DT == F32R:
            qt = _round_f32r(qt)
            vt = _round_f32r(vt)
        vn = value[b]
        if MM2_BF16:
            vn = vn.astype(ml_dtypes.bfloat16)
        vn = np.ascontiguousarray(vn)

        mask = np.zeros((NSLOT, 128, 256), dtype=np.float32)
        for j in range(NSLOT):
            t = tiles[j]
            rows = t * 128 + np.arange(128)[:, None]  # global q row
            cols = EXT[j] - 256 + np.arange(256)[None, :]  # global kv col
            mask[j][cols > rows] = -NEG_INF

        identr = np.eye(128, dtype=np.float32)

        in_maps.append(
            {"qt": qt, "vt": vt, "v": vn, "mask": mask, "identr": identr}
        )
    return in_maps


def kernel(query, value):
    nc = _get_program()
    in_maps = stage_inputs(query, value)
    res = run_bass_kernel_spmd(nc, in_maps, core_ids=list(range(NCORES)))

    out = np.empty((B, T, D), dtype=np.float32)
    for c in range(NCORES):
        o = res.results[c]["o"]  # [1024, D]
        b = c % 4
        for j, t in enumerate(_tiles_for_core(c)):
            out[b, t * 128 : (t + 1) * 128, :] = o[j * 128 : (j + 1) * 128, :]
    return out


# revision 55
# speedup vs baseline: 107.5385x; 1.0036x over previous
"""Causal dense attention (key=value) on 8 TRN2 NeuronCores.

Reference semantics (B=4, T=2048, D=1024, fp32):
    scores  = Q @ V^T                      [B, T, T]
    scores -= 1e9 * (~tril)                causal mask
    W       = softmax(scores, axis=-1)
    out     = W @ V                        [B, T, D]

Sharding: 2 cores per batch. Each batch's 16 causal q-tiles (128 rows
each, kv extent 128*(t+1)) are split odd/even so both cores get the
same padded kv-extent schedule EXT = [256, 512, ..., 2048] (ascending),
making the Bass program identical across all 8 cores (pure SPMD).
Padding columns are killed by the additive causal mask.

Host stages per core: Q^T (d-major) and V^T (d-major) pre-rounded to
f32r (full-rate TensorE), V (natural, bf16), and additive causal masks
for the last 256 columns of each slot.

Device pipeline per slot (software-pipelined, lag 2):
  mm1  S = Q^T.T @ V^T into PSUM kilo-windows (f32r), with the causal
       mask folded into the accumulation group as an identity-weighted
       matmul (S += I.T @ mask);
  stats row-max (DVE reduce, negated) -> exp with fused bias and
       accumulated row-sum (ScalarE) -> W (bf16) in SBUF;
  mm2  PE-transpose of W blocks, W^T.T @ V (bf16) into PSUM, then a
       ScalarE copy fused with the 1/rowsum scale, and DMA out.
Input DMAs are coalesced (one strided descriptor-set per wave) and
ordered by slot consumption so compute starts ~2 MiB in.
"""

import numpy as np

import concourse.bass as bass
import concourse.mybir as mybir
from concourse import bacc, tile
from concourse.bass_utils import run_bass_kernel_spmd
from concourse.masks import make_identity

import ml_dtypes

B, T, D = 4, 2048, 1024
NCORES = 8
NSLOT = 8
EXT = [256 * (j + 1) for j in range(NSLOT)]  # kv extent per slot
# processing order: smallest slot first (fast start), small slot last (short
# pipeline tail); big slots in the middle keep the PE pipeline saturated
ORDER = [1, 2, 3, 4, 5, 6, 7, 0]
NEG_INF = 1e9

F32 = mybir.dt.float32
F32R = mybir.dt.float32r
BF16 = mybir.dt.bfloat16

# matmul dtypes (tuning knobs)
MM1_DT = F32R   # scores matmul
MM2_BF16 = True  # weights/value matmul in bf16
REPEAT = 1      # >1: repeat the whole pipeline in-program (bench only)


def _round_f32r(x):
    """Round fp32 to 11 mantissa bits (RNE) — matches walrus fp32_to_fp32r."""
    u = np.asarray(x, dtype=np.float32).view(np.uint32)
    u = u + 0x7FF + ((u >> 12) & 1)
    u &= np.uint32(0xFFFFF000)
    return u.view(np.float32)


def _tiles_for_core(c):
    """q-tile index (within the batch) for each slot, for core c."""
    if c < 4:
        return [2 * j + 1 for j in range(NSLOT)]  # extents exactly EXT
    return [2 * j for j in range(NSLOT)]  # extents EXT - 128 (padded)


def _build_program():
    nc = bacc.Bacc("TRN2", target_bir_lowering=False)

    qt_d = nc.dram_tensor("qt", [D, NSLOT * 128], MM1_DT, kind="ExternalInput")
    vt_d = nc.dram_tensor("vt", [D, T], MM1_DT, kind="ExternalInput")
    v_d = nc.dram_tensor(
        "v", [T, D], BF16 if MM2_BF16 else F32, kind="ExternalInput"
    )
    mask_d = nc.dram_tensor("mask", [NSLOT, 128, 256], MM1_DT, kind="ExternalInput")
    identr_d = nc.dram_tensor("identr", [128, 128], MM1_DT, kind="ExternalInput")
    o_d = nc.dram_tensor("o", [NSLOT * 128, D], F32, kind="ExternalOutput")

    v_dt = BF16 if MM2_BF16 else F32

    with tile.TileContext(nc) as tc:
        with (
            tc.tile_pool(name="const", bufs=1) as constp,
            tc.tile_pool(name="qt", bufs=1) as qtp,
            tc.tile_pool(name="vt", bufs=1) as vtp,
            tc.tile_pool(name="vn", bufs=1) as vnp,
            tc.tile_pool(name="w", bufs=3) as wp,
            tc.tile_pool(name="wt", bufs=8) as wtp,
            tc.tile_pool(name="osb", bufs=2) as op,
            tc.tile_pool(name="stats", bufs=24) as statp,
            tc.tile_pool(name="ps_s", bufs=2, space="PSUM") as ps_s,
            tc.tile_pool(name="ps_t", bufs=2, space="PSUM") as ps_t,
            tc.tile_pool(name="ps_o", bufs=1, space="PSUM") as ps_o,
        ):
            ident = constp.tile([128, 128], BF16, tag="ident")
            make_identity(nc, ident[:])
            ident_r = constp.tile([128, 128], MM1_DT, tag="identr")

            # ACT exp-table warm-up: load exp_and_others during initial DMAs
            warm = statp.tile([128, 1], F32, tag="warm")
            nc.gpsimd.memset(warm[:], 0.0)
            nc.scalar.activation(warm[:], warm[:], mybir.ActivationFunctionType.Exp)

            masks = constp.tile([128, NSLOT * 256], MM1_DT, tag="masks")
            w_dt = BF16 if MM2_BF16 else F32

            # Resident inputs, DMA'd in slot-consumption order (per ORDER):
            # each slot first needs its Q^T quarter, V^T chunks and V rows up
            # to its extent; the masks ride along after the first slot's data.
            qts = {}   # (d8, j) -> [128, 128] AP
            vts = {}   # (d8, kc) -> [128, 512]
            vns = {}   # kb -> [128, D]

            def emit_dma_waves(j, first_rep):
                # one coalesced DMA per wave: SBUF [128, 8, w] <- DRAM
                # [(8*128), w] with d8 stacked along the free dim
                if (0, j) not in qts:
                    # early waves small (fast pipeline start), later 512-wide
                    if j < 2:
                        c0, cw = 0, 256
                    elif j < 4:
                        c0, cw = 256, 256
                    else:
                        c0, cw = 512, 512
                    t_ = qtp.tile([128, 8, cw], MM1_DT, tag=f"qtw{c0}")
                    nc.sync.dma_start(
                        t_[:], qt_d[:, c0 : c0 + cw].rearrange("(a p) q -> p a q", p=128)
                    )
                    for d8 in range(8):
                        for jj in range(c0 // 128, (c0 + cw) // 128):
                            qts[(d8, jj)] = t_[
                                :, d8,
                                (jj - c0 // 128) * 128 : (jj - c0 // 128 + 1) * 128,
                            ]
                for kc in range((EXT[j] + 511) // 512):
                    if (0, kc) in vts:
                        continue
                    t_ = vtp.tile([128, 8, 512], MM1_DT, tag=f"vtw{kc}")
                    if kc == 0:
                        # split the first chunk so the opening slot's
                        # matmuls start after ~1MiB instead of 2MiB
                        for hh in (0, 256):
                            nc.sync.dma_start(
                                t_[:, :, hh : hh + 256],
                                vt_d[:, hh : hh + 256].rearrange(
                                    "(a p) k -> p a k", p=128
                                ),
                            )
                    else:
                        nc.sync.dma_start(
                            t_[:],
                            vt_d[:, kc * 512 : (kc + 1) * 512].rearrange(
                                "(a p) k -> p a k", p=128
                            ),
                        )
                    for d8 in range(8):
                        vts[(d8, kc)] = t_[:, d8, :]
                if first_rep and j == ORDER[0]:
                    # needed only at the tail of the first mm1 group — keep
                    # these off the head of the DMA queue
                    nc.sync.dma_start(
                        masks[:].rearrange("p (j c) -> p j c", j=NSLOT),
                        mask_d.rearrange("j p c -> p j c"),
                    )
                    nc.sync.dma_start(ident_r[:], identr_d[:])

            def emit_front(j):
                """mm1 + softmax stats + exp for slot j. Returns (j, w_sb, rinv)."""
                E = EXT[j]
                # kilo-windows of up to 1024 columns (each a [128,1024] PSUM
                # tile = 2 banks; matmuls still issue per 512-column bank)
                kws = [(c0, min(1024, E - c0)) for c0 in range(0, E, 1024)]

                # sub-pass width: the opening slot streams 256-wide so its
                # first matmuls only wait on the first 1MiB V^T half-DMA
                sw_ = 256 if j == ORDER[0] else 512

                s_tiles = []
                for c0, kwd in kws:
                    s_ = ps_s.tile([128, 1024], F32, tag="sw")
                    for h0 in range(0, kwd, 512):
                        hw = min(512, kwd - h0)
                        a0 = c0 + h0  # absolute column
                        # the additive causal mask (last 256 columns of the
                        # slot) rides the accumulation group as an extra
                        # identity-weighted matmul: S[q,c] += I[k,q]*mask[k,c]
                        last_half = c0 + h0 + hw == E
                        for g0 in range(0, hw, sw_):
                            gw = min(sw_, hw - g0)
                            for d8 in range(8):
                                nc.tensor.matmul(
                                    s_[:, h0 + g0 : h0 + g0 + gw],
                                    qts[(d8, j)],
                                    vts[(d8, a0 // 512)][
                                        :,
                                        a0 % 512 + g0 : a0 % 512 + g0 + gw,
                                    ],
                                    start=(d8 == 0 and g0 == 0),
                                    stop=(
                                        d8 == 7
                                        and g0 + gw == hw
                                        and not last_half
                                    ),
                                )
                        if last_half:
                            nc.tensor.matmul(
                                s_[:, h0 + hw - 256 : h0 + hw],
                                ident_r[:],
                                masks[:, j * 256 : (j + 1) * 256],
                                start=False,
                                stop=True,
                            )
                    s_tiles.append(s_)

                # negated row max over the strip
                nmax = None
                for ki, (c0, kwd) in enumerate(kws):
                    nm = statp.tile([128, 1], F32, tag="nm")
                    nc.vector.reduce_max(
                        nm[:], s_tiles[ki][:, :kwd], axis=mybir.AxisListType.X,
                        negate=True,
                    )
                    if nmax is None:
                        nmax = nm
                    else:
                        nm2 = statp.tile([128, 1], F32, tag="nmc")
                        nc.vector.tensor_tensor(
                            nm2[:], nmax[:], nm[:], op=mybir.AluOpType.min
                        )
                        nmax = nm2

                # exp (fused bias & row-sum) -> W in SBUF
                w_sb = wp.tile([128, E], w_dt, tag="w")
                rsum = None
                for ki, (c0, kwd) in enumerate(kws):
                    rs = statp.tile([128, 1], F32, tag="rs")
                    nc.scalar.activation(
                        w_sb[:, c0 : c0 + kwd],
                        s_tiles[ki][:, :kwd],
                        mybir.ActivationFunctionType.Exp,
                        bias=nmax[:],
                        accum_out=rs[:],
                    )
                    if rsum is None:
                        rsum = rs
                    else:
                        rs2 = statp.tile([128, 1], F32, tag="rsc")
                        nc.vector.tensor_add(rs2[:], rsum[:], rs[:])
                        rsum = rs2
                rinv = statp.tile([128, 1], F32, tag="rinv")
                nc.vector.reciprocal(rinv[:], rsum[:])
                return (j, w_sb, rinv)

            def emit_vn_waves(j):
                for q_ in range((EXT[j] // 128 + 3) // 4):
                    if q_ * 4 in vns:
                        continue
                    t_ = vnp.tile([128, 4, D], v_dt, tag=f"vnw{q_}")
                    nc.sync.dma_start(
                        t_[:],
                        v_d[q_ * 512 : (q_ + 1) * 512, :].rearrange(
                            "(a p) d -> p a d", p=128
                        ),
                    )
                    for kb in range(q_ * 4, q_ * 4 + 4):
                        vns[kb] = t_[:, kb % 4, :]

            def emit_back(state):
                """transpose W, mm2, normalize, store for a finished slot."""
                j, w_sb, rinv = state
                E = EXT[j]
                nblk = E // 128
                wt_tiles = []
                for g0 in range(0, nblk, 4):
                    gn = min(4, nblk - g0)
                    t_ps = ps_t.tile([128, 512], w_dt, tag="tp")
                    for bi in range(gn):
                        blk = g0 + bi
                        nc.tensor.transpose(
                            t_ps[:, bi * 128 : (bi + 1) * 128],
                            w_sb[:, blk * 128 : (blk + 1) * 128],
                            ident[:],
                        )
                    wt_sb = wtp.tile([128, 512], w_dt, tag="wt")
                    nc.vector.tensor_copy(wt_sb[:, : gn * 128], t_ps[:, : gn * 128])
                    wt_tiles.append(wt_sb)

                # output matmul: O[q, d] = W^T.T @ V
                o_ps = ps_o.tile([128, D], F32, tag="op")
                for blk in range(nblk):
                    wt_ap = wt_tiles[blk // 4][
                        :, (blk % 4) * 128 : (blk % 4 + 1) * 128
                    ]
                    v_ap = vns[blk]
                    if not MM2_BF16:
                        wt_ap = wt_ap.bitcast(F32R)
                    for dd in range(0, D, 512):
                        va = v_ap[:, dd : dd + 512]
                        if not MM2_BF16:
                            va = va.bitcast(F32R)
                        nc.tensor.matmul(
                            o_ps[:, dd : dd + 512],
                            wt_ap,
                            va,
                            start=(blk == 0),
                            stop=(blk == nblk - 1),
                        )

                o_sb = op.tile([128, D], F32, tag="o")
                for dd in range(0, D, 512):
                    nc.scalar.activation(
                        o_sb[:, dd : dd + 512],
                        o_ps[:, dd : dd + 512],
                        mybir.ActivationFunctionType.Copy,
                        scale=rinv[:],
                    )
                    nc.sync.dma_start(
                        o_d[j * 128 : (j + 1) * 128, dd : dd + 512],
                        o_sb[:, dd : dd + 512],
                    )

            # software pipeline (lag 2): slot j's scores/softmax overlap the
            # two previous slots' transpose+mm2 work queued on the PE
            pending = []
            for rep in range(REPEAT):
                if rep > 0:
                    # drain the pipeline before re-loading inputs (bench path)
                    for st in pending:
                        emit_back(st)
                    pending = []
                    qts.clear()
                    vts.clear()
                    vns.clear()
                for j in ORDER:
                    emit_dma_waves(j, rep == 0)
                    pending.append(emit_front(j))
                    emit_vn_waves(j)
                    if len(pending) > 2:
                        emit_back(pending.pop(0))
            for st in pending:
                emit_back(st)

    nc.finalize()
    return nc


_NC_CACHE = None


def _get_program():
    global _NC_CACHE
    if _NC_CACHE is None:
        _NC_CACHE = _build_program()
    return _NC_CACHE


def stage_inputs(query, value):
    """Build the 8 per-core input maps from the full query/value arrays."""
    query = np.asarray(query, dtype=np.float32)
    value = np.asarray(value, dtype=np.float32)

    in_maps = []
    for c in range(NCORES):
        b = c % 4
        tiles = _tiles_for_core(c)

        q_rows = np.concatenate(
            [query[b, t * 128 : (t + 1) * 128, :] for t in tiles], axis=0
        )  # [1024, D]
        qt = np.ascontiguousarray(q_rows.T)  # [D, 1024]
        vt = np.ascontiguousarray(value[b].T)  # [D, T]
        if MM1_DT == F32R:
            qt = _round_f32r(qt)
            vt = _round_f32r(vt)
        vn = value[b]
        if MM2_BF16:
            vn = vn.astype(ml_dtypes.bfloat16)
        vn = np.ascontiguousarray(vn)

        mask = np.zeros((NSLOT, 128, 256), dtype=np.float32)
        for j in range(NSLOT):
            t = tiles[j]
            rows = t * 128 + np.arange(128)[:, None]  # global q row
            cols = EXT[j] - 256 + np.arange(256)[None, :]  # global kv col
            mask[j][cols > rows] = -NEG_INF

        identr = np.eye(128, dtype=np.float32)

        in_maps.append(
            {"qt": qt, "vt": vt, "v": vn, "mask": mask, "identr": identr}
        )
    return in_maps


def kernel(query, value):
    nc = _get_program()
    in_maps = stage_inputs(query, value)
    res = run_bass_kernel_spmd(nc, in_maps, core_ids=list(range(NCORES)))

    out = np.empty((B, T, D), dtype=np.float32)
    for c in range(NCORES):
        o = res.results[c]["o"]  # [1024, D]
        b = c % 4
        for j, t in enumerate(_tiles_for_core(c)):
            out[b, t * 128 : (t + 1) * 128, :] = o[j * 128 : (j + 1) * 128, :]
    return out
